# revision 80
# baseline (speedup 1.0000x reference)
"""Trainium2 Bass kernel: batched multi-head attention softmax(Q K^T) V.

Full inputs: q/k/v [4, 16, 2048, 64] f32. Sharded over 8 NeuronCores by
flattened (batch, head): core i computes heads [8i, 8i+8).

Per-head algorithm (S=2048, D=64, P=128):
  - Q,K,V loaded bf16 (casting SWDGE DMA), Q^T/K^T built d-major via XBAR
    DMA-transposes of [128,128] bf16 tiles (striped s-order, tracked in
    index math).
  - scores^T tiles [128 t, 1024 s] on TensorE (bf16, fp32 PSUM), exp split
    across ScalarE (native Exp) and VectorE (1-op Schraudolph bit-trick:
    bf16 bits = int16(x*128/ln2 + 127*128 - C), exact enough because softmax
    renormalization cancels the common-mode error). No max subtraction:
    |scores| < ~50 so exp fits fp32/bf16.
  - O^T[65, 2048] = sum_t V_aug[t]^T @ E[t] accumulated in PSUM, where
    V_aug has a ones column => row 64 = softmax denominators.
  - finish: O^T copied to SBUF bf16 (ACT+DVE halves; frees the OT PSUM slot
    within ~1 unit so the next head's accumulation starts on time), XBAR
    DMA-transposed to s-major, normalized by 1/denominator on GpSimd,
    DMA out fp32.
"""

import os
import sys
import numpy as np

_TRN_REPO = "/opt/trn_rl_repo"

B, H, S, D = 4, 16, 2048, 64
P = 128
N_CORES = 8
HEADS = (B * H) // N_CORES  # heads per core
TB = S // P  # 16 t-blocks

_prog_cache = {}
PHASE_MARKS = []


def _s_start(j):
    """DRAM s-offset of the 128-row block behind psum column block j.

    OT psum columns are ordered chunk-major: chunk c in [0,4) of 512 cols,
    within chunk 4 m-blocks of 128. Column (c, mm, r) holds
    s = 256*m + 128*bq + r with bq = c//2, m = 4*(c%2) + mm.
    """
    c, mm = j // 4, j % 4
    return 256 * (4 * (c % 2) + mm) + 128 * (c // 2)


# Schraudolph fast-exp constants (bf16 bits via int16 convert).
# exp(x) ~ bf16_bits( int16( x * 2^7/ln2 + (127*2^7 - C) ) ); C balances the
# (1+f) vs 2^f interpolation error. Constant scale offsets cancel in softmax.
SCHR_A = 184.66496523378733
SCHR_B = 16256.0 - 4.5


def _build_program(heads=HEADS, dumps=False, reps=1, ot_lag=6, ebufs=10):
    if _TRN_REPO not in sys.path:
        sys.path.insert(0, _TRN_REPO)
    import concourse.bacc as bacc
    import concourse.mybir as mybir
    import concourse.tile as tile
    from bass_rust import add_dep_helper
    from contextlib import ExitStack

    f32 = mybir.dt.float32
    bf16 = mybir.dt.bfloat16
    i16 = mybir.dt.int16
    EXP = mybir.ActivationFunctionType.Exp
    MULT = mybir.AluOpType.mult
    ADD = mybir.AluOpType.add
    DIV = mybir.AluOpType.divide

    nc = bacc.Bacc("TRN2", target_bir_lowering=False, debug=False)
    q_d = nc.declare_dram_parameter("q", [heads, S, D], f32, isOutput=False)
    k_d = nc.declare_dram_parameter("k", [heads, S, D], f32, isOutput=False)
    v_d = nc.declare_dram_parameter("v", [heads, S, D], f32, isOutput=False)
    o_d = nc.declare_dram_parameter("out", [heads, S, D], f32, isOutput=True)
    dump_d = {}
    if dumps:
        for nm, shape, dt_ in [
            ("qT_dump", [P, TB // 2, P], bf16),
            ("kT_dump", [P, TB // 2, P], bf16),
            ("kTs_dump", [P, TB // 2 + 1, P], bf16),
            ("e_dump", [TB, P, S], bf16),
        ]:
            dump_d[nm] = nc.declare_dram_parameter(nm, shape, dt_, isOutput=True)

    with tile.TileContext(nc) as tc, ExitStack() as ctx:
        pool = lambda name, bufs, **kw: ctx.enter_context(
            tc.tile_pool(name=name, bufs=bufs, **kw)
        )
        const_pool = pool("const", 1)
        qbf_pool = pool("qbf", 5)
        kbf_pool = pool("kbf", 5)
        vaug_pool = pool("vaug", 5)
        qT_pool = pool("qT", 5)
        kT_pool = pool("kT", 5)
        kTs_pool = pool("kTs", 5)
        # One e-tile per producing engine per score quarter: sharing a tile
        # between ACT and DVE writers would serialize them in the tracker.
        e_act_pools = [pool("eA0", ebufs), pool("eA1", ebufs)]
        e_dve_pools = [pool("eD0", ebufs), pool("eD1", ebufs)]
        osb_pool = pool("osb", 4)
        osT_pool = pool("osT", 4)
        den_pool = pool("den", 4)
        obuf_pool = pool("obuf", 2)
        obufb_pool = pool("obufb", 2)
        # One single-writer single-reader PSUM bank per 512-col score matmul:
        # the tile tracker serializes ALL accessors of a psum tile, so any
        # sharing puts one exp engine behind the other.
        psQ = [pool(f"psQ{i}", 1, space="PSUM") for i in range(4)]
        # O^T accumulator in two column halves: the tile tracker serializes
        # all accessors of a tile, so separate halves let the two freeing
        # copies (ACT + DVE) run in parallel at the head boundary.
        psOTa = pool("psOTa", 1, space="PSUM")
        psOTb = pool("psOTb", 1, space="PSUM")

        warm = const_pool.tile([P, 1], f32)
        # O^T staging for the XBAR transpose: [80, 1024] bf16 per column half
        # (65 data rows, 80 = multiple of XBAR_TILE_SRC_ROWS). Rows 65:80 are
        # zeroed once here and never written again, so the transposed junk
        # columns are finite and ignored. Separate tiles per half so the ACT
        # and DVE copies are independent; two buffers each, alternating.
        # Static K staging / V_aug tiles: the zero padding blocks (k_bf) and
        # the ones column (v_aug) are initialized once; per-head loads only
        # rewrite the data regions, keeping memsets out of the load chain.
        # k_bf edge memsets come first (tile 0's gate the first kT/kTs
        # transposes); the big osb/v_aug memsets follow.
        kbf_tiles = []
        vaug_tiles = []
        for _ in range(5):
            t = kbf_pool.tile([P, TB + 2, D], bf16)
            nc.vector.memset(t[:, 0, :], 0.0)
            nc.vector.memset(t[:, TB + 1, :], 0.0)
            kbf_tiles.append(t)
        osb_tiles = []  # [half][buf]
        for _ in range(2):
            bufs = []
            for _ in range(2):
                t = osb_pool.tile([80, S // 2], bf16)
                # partition base must be 32-aligned; row 64 (denominators)
                # is rewritten by every head's copy
                nc.vector.memset(t[D : 80, :], 0.0)
                bufs.append(t)
            osb_tiles.append(bufs)
        for _ in range(heads):
            t2 = vaug_pool.tile([P, TB, D + 1], bf16)
            # only the ones column; data columns are loaded before any read
            nc.vector.memset(t2[:, :, D], 1.0)
            vaug_tiles.append(t2)

        pend = {}  # head -> (q_bf, k_bf, v_aug, qT, kT, kTs) ready

        last_qT_tr = [None]  # previous head's last transpose (pacing anchor)
        last_v_ld = [None]  # previous head's V load (scheduler order anchor)

        def issue_loads(hd):
            PHASE_MARKS.append((nc.next_id(), f"loads_h{hd}"))
            # K first (its transposes gate the first score matmul), V last.
            # Pace copy issue so the DMA queue never jams the XBAR path:
            # loads of head h+1 and this head's V load go behind this head's
            # transposes via explicit dep edges.
            # K staging padded by one 64-col zero block on each side so the
            # shifted transposes below can cover edge t-blocks.
            k_bf = kbf_tiles[hd % len(kbf_tiles)]
            k_ld = nc.gpsimd.dma_start(
                out=k_bf[:, 1 : TB + 1, :],
                in_=k_d[hd % heads].rearrange("(n p) d -> p n d", p=P),
            )
            q_bf = qbf_pool.tile([P, TB, D], bf16)
            q_ld = nc.gpsimd.dma_start(
                out=q_bf[:], in_=q_d[hd % heads].rearrange("(n p) d -> p n d", p=P)
            )
            if last_qT_tr[0] is not None:
                add_dep_helper(k_ld.ins, last_qT_tr[0],
                               reason="pace loads behind prev transposes")
                add_dep_helper(q_ld.ins, last_qT_tr[0],
                               reason="pace loads behind prev transposes")
            # V loads for the first pass all go out upfront (see the g==0
            # block): any V load sitting behind a paced K load in the SWDGE
            # queue gets blocked by the hoisted pacing wait and starves the
            # OT matmuls during ramp-up. In reps mode, re-issue per head
            # (same data) to keep the measured steady state honest.
            v_aug = vaug_tiles[hd % heads]
            if hd >= heads:
                v_ld = nc.gpsimd.dma_start(
                    out=v_aug[:, :, 0:D],
                    in_=v_d[hd % heads].rearrange("(n p) d -> p n d", p=P),
                )
                add_dep_helper(v_ld.ins, last_qT_tr[0],
                               reason="steady-state v after transposes")
            # Natural transposes: slot m has t-block 2m on partitions 0-63 and
            # t-block 2m+1 on partitions 64-127 (k_bf block i holds t-block i-1).
            kT = kT_pool.tile([P, TB // 2, P], bf16)
            nc.sync.dma_start(out=kT[:], in_=k_bf[:, 1 : TB + 1, :], transpose=True)
            # Shifted transposes: slot m has t-block 2m-1 on partitions 0-63
            # and t-block 2m on partitions 64-127 (junk/zero at the edges).
            kTs = kTs_pool.tile([P, TB // 2 + 1, P], bf16)
            nc.sync.dma_start(out=kTs[:], in_=k_bf[:, 0 : TB + 2, :], transpose=True)
            # Batched xbar transposes: out[:, m, :] = in[:, 128m:128(m+1)].T
            qT = qT_pool.tile([P, TB // 2, P], bf16)
            qT_tr = nc.sync.dma_start(out=qT[:], in_=q_bf[:], transpose=True)
            last_qT_tr[0] = qT_tr.ins
            if dumps and hd == 0:
                nc.sync.dma_start(out=dump_d["qT_dump"][:], in_=qT[:])
                nc.sync.dma_start(out=dump_d["kT_dump"][:], in_=kT[:])
                nc.sync.dma_start(out=dump_d["kTs_dump"][:], in_=kTs[:])
            pend[hd] = (q_bf, k_bf, v_aug, qT, kT, kTs)

        def kt_block(kT, kTs, tb, bq):
            """lhsT [64, 128] for t-block tb based at partition 64*bq."""
            lo = 64 * bq
            if bq == tb % 2:
                return kT[lo : lo + 64, tb // 2, :]
            if bq == 0:  # tb odd: shifted slot (tb+1)//2, lower half
                return kTs[0:64, (tb + 1) // 2, :]
            return kTs[64:128, tb // 2, :]  # tb even, upper half

        OT_LAG = ot_lag
        TAIL_LAG = 11  # next-head score-unit where the normalize tail begins

        def emit_ot(ot_ab, v_aug, e_tiles, tb):
            vt = v_aug[:, tb, :]
            ets = e_tiles.pop(tb)
            for c in range(4):
                nc.tensor.matmul(
                    ot_ab[c // 2][:, 512 * (c % 2) : 512 * (c % 2 + 1)],
                    lhsT=vt,
                    rhs=ets[c][:],
                    start=(tb == 0),
                    stop=(tb == TB - 1),
                )

        fin_pend = {}  # head -> osT tile(s) awaiting the normalize tail

        def finish_front(hd, ot_ab, last=False):
            """O^T psum halves -> SBUF bf16 -> XBAR transpose to s-major.

            Half A copied by ACT, half B by DVE, in parallel: psOTa/b are
            freed within ~1 unit so the next head's OT accumulation starts
            on time. O^T transits bf16, which costs ~0.4% on output and the
            denominators - fine at 2e-2.
            """
            PHASE_MARKS.append((nc.next_id(), f"finfront_h{hd}"))
            osTs = []
            for h2 in range(2):
                osb = osb_tiles[h2][hd % 2]
                if h2 == 0:
                    nc.scalar.copy(osb[0 : D + 1, :], ot_ab[h2][:])
                else:
                    nc.vector.tensor_copy(osb[0 : D + 1, :], ot_ab[h2][:])
                osT = osT_pool.tile([P, TB // 2, 80], bf16)
                nc.sync.dma_start(out=osT[:], in_=osb[:], transpose=True)
                osTs.append(osT)
            fin_pend[hd] = osTs

        def finish_tail_start(hd):
            """Reciprocals of the denominators (osT col 64) on DVE."""
            PHASE_MARKS.append((nc.next_id(), f"fintail_h{hd}"))
            osT_a, osT_b = fin_pend[hd]
            rec_a = den_pool.tile([P, TB // 2], f32)
            nc.vector.reciprocal(rec_a[:], osT_a[:, :, D])
            rec_b = den_pool.tile([P, TB // 2], f32)
            nc.vector.reciprocal(rec_b[:], osT_b[:, :, D])
            obuf_a = obuf_pool.tile([P, 8, D], f32)
            obuf_b = obufb_pool.tile([P, 8, D], f32)
            fin_pend[hd] = (osT_a, osT_b, rec_a, rec_b, obuf_a, obuf_b)

        def finish_tail_piece(hd, jj):
            """Normalize block jj of each half: ACT (Copy with per-partition
            scale AP) for half A, DVE for half B. One piece per unit keeps
            the normalize work from bursting ahead of exps in the queues;
            separate obuf tiles avoid accessor serialization."""
            osT_a, osT_b, rec_a, rec_b, obuf_a, obuf_b = fin_pend[hd]
            nc.scalar.mul(
                obuf_a[:, jj, :], osT_a[:, jj, 0:D], rec_a[:, jj : jj + 1]
            )
            nc.vector.tensor_scalar_mul(
                obuf_b[:, jj, :], osT_b[:, jj, 0:D], rec_b[:, jj : jj + 1]
            )

        def finish_tail_store(hd):
            _, _, _, _, obuf_a, obuf_b = fin_pend.pop(hd)
            dst = o_d[hd % heads].rearrange("(m b p) d -> p m b d", m=8, b=2, p=P)
            nc.sync.dma_start(out=dst[:, :, 0, :], in_=obuf_a[:])
            nc.sync.dma_start(out=dst[:, :, 1, :], in_=obuf_b[:])

        # Global software-pipelined stream over (head, t-block) units.
        # Scores+exp for unit g are emitted at step g; the OT accumulation for
        # unit g-OT_LAG follows, so the OT tail of head h interleaves with the
        # first score blocks of head h+1 and ACT never waits on it.
        heads_ctx = {}  # head -> (v_aug, e_tiles, ot)
        PREFETCH = 3
        total = heads * reps

        def emit_unit(g):
            hd, tb = divmod(g, TB)
            if tb == 0:
                PHASE_MARKS.append((nc.next_id(), f"score_h{hd}"))
                _q_bf, _k_bf, v_aug, qT, kT, kTs = pend.pop(hd)
                heads_ctx[hd] = {"v": v_aug, "qT": qT, "kT": kT, "kTs": kTs,
                                 "e": {}, "ot": None}
            ctx_h = heads_ctx[hd]
            qT, kT, kTs = ctx_h["qT"], ctx_h["kT"], ctx_h["kTs"]
            # Exp is split across ScalarE (native) and VectorE (Schraudolph),
            # each issued right after the 512-col matmul filling its psum
            # bank, so the psum WAR chains resolve well before the next
            # unit's score matmuls need the banks.
            e_tiles = []
            for q4 in range(4):
                bq, g2 = divmod(q4, 2)
                st = psQ[q4].tile([P, 512], f32)
                nc.tensor.matmul(
                    st[:],
                    lhsT=kt_block(kT, kTs, tb, bq),
                    rhs=qT[64 * bq : 64 * bq + 64, 4 * g2 : 4 * g2 + 4, :],
                    start=True,
                    stop=True,
                )
                if g2 == 0:  # ScalarE native exp
                    et = e_act_pools[bq].tile([P, 512], bf16)
                    nc.scalar.activation(et[:], st[:], EXP)
                else:  # VectorE Schraudolph
                    et = e_dve_pools[bq].tile([P, 512], bf16)
                    nc.vector.tensor_scalar(
                        et[:].bitcast(i16), st[:], SCHR_A, SCHR_B, MULT, ADD,
                    )
                e_tiles.append(et)
            if dumps and hd == 0:
                for q4 in range(4):
                    nc.sync.dma_start(
                        out=dump_d["e_dump"][tb, :, 512 * q4 : 512 * (q4 + 1)],
                        in_=e_tiles[q4][:],
                    )
            ctx_h["e"][tb] = e_tiles
            if hd >= 1 and (hd - 1) in fin_pend:
                if tb == TAIL_LAG:
                    finish_tail_start(hd - 1)
                elif TAIL_LAG < tb <= TAIL_LAG + 4:
                    jj0 = 2 * (tb - TAIL_LAG - 1)
                    finish_tail_piece(hd - 1, jj0)
                    finish_tail_piece(hd - 1, jj0 + 1)
                    if tb == TAIL_LAG + 4:
                        finish_tail_store(hd - 1)

        def emit_ot_unit(g):
            hd, tb = divmod(g, TB)
            ctx_h = heads_ctx[hd]
            if tb == 0:
                ot_a = psOTa.tile([D + 1, S // 2], f32, tag="otslotA")
                ot_b = psOTb.tile([D + 1, S // 2], f32, tag="otslotB")
                ctx_h["ot"] = (ot_a, ot_b)
            emit_ot(ctx_h["ot"], ctx_h["v"], ctx_h["e"], tb)
            if tb == TB - 1:
                finish_front(hd, ctx_h["ot"], last=(hd == total - 1))
                heads_ctx.pop(hd)

        n_units = total * TB
        for g in range(n_units + OT_LAG):
            gh = g // TB
            if g == 0:
                issue_loads(0)
                # All first-pass V tensors upfront, before any paced K load
                # enters the SWDGE queue (a hoisted pacing wait would block
                # them and starve the OT matmuls during ramp-up).
                for vh in range(heads):
                    nc.gpsimd.dma_start(
                        out=vaug_tiles[vh][:, :, 0:D],
                        in_=v_d[vh].rearrange("(n p) d -> p n d", p=P),
                    )
                for pf in range(1, min(PREFETCH, total)):
                    issue_loads(pf)
                # warm the exp table set while the first input DMAs are in
                # flight
                nc.vector.memset(warm[:], 0.0)
                nc.scalar.activation(warm[:], warm[:], EXP)
            elif g % TB == 0 and gh + PREFETCH - 1 < total:
                issue_loads(gh + PREFETCH - 1)
            if g < n_units:
                emit_unit(g)
            if g >= OT_LAG:
                emit_ot_unit(g - OT_LAG)
        # last head's tail: nothing left to interleave with, emit as a burst
        finish_tail_start(total - 1)
        for jj in range(TB // 2):
            finish_tail_piece(total - 1, jj)
        finish_tail_store(total - 1)

    PHASE_MARKS.append((nc.next_id(), "END"))
    if not nc.is_finalized():
        nc.finalize()
    return nc


def _get_program():
    if "nc" not in _prog_cache:
        _prog_cache["nc"] = _build_program()
    return _prog_cache["nc"]


def _run(in_maps, trace=False):
    if _TRN_REPO not in sys.path:
        sys.path.insert(0, _TRN_REPO)
    from concourse.bass_utils import run_bass_kernel_spmd

    nc = _get_program()
    return run_bass_kernel_spmd(nc, in_maps, list(range(N_CORES)), trace=trace)


def _make_in_maps(input_query, input_key, input_value):
    q = np.ascontiguousarray(np.asarray(input_query, np.float32)).reshape(B * H, S, D)
    k = np.ascontiguousarray(np.asarray(input_key, np.float32)).reshape(B * H, S, D)
    v = np.ascontiguousarray(np.asarray(input_value, np.float32)).reshape(B * H, S, D)
    in_maps = []
    for i in range(N_CORES):
        sl = slice(i * HEADS, (i + 1) * HEADS)
        in_maps.append(
            {
                "q": np.ascontiguousarray(q[sl]),
                "k": np.ascontiguousarray(k[sl]),
                "v": np.ascontiguousarray(v[sl]),
            }
        )
    return in_maps


def kernel(input_query, input_key, input_value):
    in_maps = _make_in_maps(input_query, input_key, input_value)
    res = _run(in_maps, trace=False)
    out = np.concatenate([np.asarray(r["out"]) for r in res.results], axis=0)
    return out.reshape(B, H, S, D).astype(np.float32)


def kernel_traced(input_query, input_key, input_value):
    """Like kernel() but with neuron-profile tracing; returns (out, results)."""
    in_maps = _make_in_maps(input_query, input_key, input_value)
    res = _run(in_maps, trace=True)
    out = np.concatenate([np.asarray(r["out"]) for r in res.results], axis=0)
    return out.reshape(B, H, S, D).astype(np.float32), res



# revision 83
# speedup vs baseline: 1.3052x; 1.3052x over previous
"""Trainium2 Bass kernel: batched multi-head attention softmax(Q K^T) V.

Full inputs: q/k/v [4, 16, 2048, 64] f32. Sharded over 8 NeuronCores by
flattened (batch, head): core i computes heads [8i, 8i+8).

Per-head algorithm (S=2048, D=64, P=128):
  - Q,K,V loaded bf16 (casting SWDGE DMA), Q^T/K^T built d-major via XBAR
    DMA-transposes of [128,128] bf16 tiles (striped s-order, tracked in
    index math).
  - scores^T tiles [128 t, 1024 s] on TensorE (bf16, fp32 PSUM), exp split
    across ScalarE (native Exp) and VectorE (1-op Schraudolph bit-trick:
    bf16 bits = int16(x*128/ln2 + 127*128 - C), exact enough because softmax
    renormalization cancels the common-mode error). No max subtraction:
    |scores| < ~50 so exp fits fp32/bf16.
  - O^T[65, 2048] = sum_t V_aug[t]^T @ E[t] accumulated in PSUM, where
    V_aug has a ones column => row 64 = softmax denominators.
  - finish: O^T copied to SBUF bf16 (ACT+DVE halves; frees the OT PSUM slot
    within ~1 unit so the next head's accumulation starts on time), XBAR
    DMA-transposed to s-major, normalized by 1/denominator on GpSimd,
    DMA out fp32.
"""

import os
import sys
import numpy as np

_TRN_REPO = "/opt/trn_rl_repo"

B, H, S, D = 4, 16, 2048, 64
P = 128
N_CORES = 8
HEADS = (B * H) // N_CORES  # heads per core
TB = S // P  # 16 t-blocks

_prog_cache = {}
PHASE_MARKS = []


def _s_start(j):
    """DRAM s-offset of the 128-row block behind psum column block j.

    OT psum columns are ordered chunk-major: chunk c in [0,4) of 512 cols,
    within chunk 4 m-blocks of 128. Column (c, mm, r) holds
    s = 256*m + 128*bq + r with bq = c//2, m = 4*(c%2) + mm.
    """
    c, mm = j // 4, j % 4
    return 256 * (4 * (c % 2) + mm) + 128 * (c // 2)


# Schraudolph fast-exp constants (bf16 bits via int16 convert).
# exp(x) ~ bf16_bits( int16( x * 2^7/ln2 + (127*2^7 - C) ) ); C balances the
# (1+f) vs 2^f interpolation error. Constant scale offsets cancel in softmax.
SCHR_A = 184.66496523378733
SCHR_B = 16256.0 - 4.5


def _build_program(heads=HEADS, dumps=False, reps=1, ot_lag=6, ebufs=10):
    if _TRN_REPO not in sys.path:
        sys.path.insert(0, _TRN_REPO)
    import concourse.bacc as bacc
    import concourse.mybir as mybir
    import concourse.tile as tile
    from bass_rust import add_dep_helper
    from contextlib import ExitStack

    f32 = mybir.dt.float32
    bf16 = mybir.dt.bfloat16
    i16 = mybir.dt.int16
    EXP = mybir.ActivationFunctionType.Exp
    MULT = mybir.AluOpType.mult
    ADD = mybir.AluOpType.add

    nc = bacc.Bacc("TRN2", target_bir_lowering=False, debug=False)
    q_d = nc.declare_dram_parameter("q", [heads, S, D], f32, isOutput=False)
    k_d = nc.declare_dram_parameter("k", [heads, S, D], f32, isOutput=False)
    v_d = nc.declare_dram_parameter("v", [heads, S, D], f32, isOutput=False)
    o_d = nc.declare_dram_parameter("out", [heads, S, D], f32, isOutput=True)
    dump_d = {}
    if dumps:
        for nm, shape, dt_ in [
            ("qT_dump", [P, TB // 2, P], bf16),
            ("kT_dump", [P, TB // 2, P], bf16),
            ("kTs_dump", [P, TB // 2 + 1, P], bf16),
            ("e_dump", [TB, P, S], bf16),
        ]:
            dump_d[nm] = nc.declare_dram_parameter(nm, shape, dt_, isOutput=True)

    with tile.TileContext(nc) as tc, ExitStack() as ctx:
        pool = lambda name, bufs, **kw: ctx.enter_context(
            tc.tile_pool(name=name, bufs=bufs, **kw)
        )
        const_pool = pool("const", 1)
        qbf_pool = pool("qbf", 5)
        kbf_pool = pool("kbf", 5)
        vaug_pool = pool("vaug", heads)
        qT_pool = pool("qT", 5)
        kT_pool = pool("kT", 5)
        kTs_pool = pool("kTs", 5)
        # One e-tile per producing engine per score quarter: sharing a tile
        # between ACT and DVE writers would serialize them in the tracker.
        e_act_pools = [pool("eA0", ebufs), pool("eA1", ebufs)]
        e_dve_pools = [pool("eD0", ebufs), pool("eD1", ebufs)]
        osb_pool = pool("osb", 4)
        osT_pool = pool("osT", 4)
        den_pool = pool("den", 4)
        obuf_pool = pool("obuf", 2)
        obufb_pool = pool("obufb", 2)
        # One single-writer single-reader PSUM bank per 512-col score matmul:
        # the tile tracker serializes ALL accessors of a psum tile, so any
        # sharing puts one exp engine behind the other.
        psQ = [pool(f"psQ{i}", 1, space="PSUM") for i in range(4)]
        # O^T accumulator in two column halves: the tile tracker serializes
        # all accessors of a tile, so separate halves let the two freeing
        # copies (ACT + DVE) run in parallel at the head boundary.
        psOTa = pool("psOTa", 1, space="PSUM")
        psOTb = pool("psOTb", 1, space="PSUM")

        warm = const_pool.tile([P, 1], f32)
        # O^T staging for the XBAR transpose: [80, 1024] bf16 per column half
        # (65 data rows, 80 = multiple of XBAR_TILE_SRC_ROWS). Rows 65:80 are
        # zeroed once here and never written again, so the transposed junk
        # columns are finite and ignored. Separate tiles per half so the ACT
        # and DVE copies are independent; two buffers each, alternating.
        # Static K staging / V_aug tiles: the zero padding blocks (k_bf) and
        # the ones column (v_aug) are initialized once; per-head loads only
        # rewrite the data regions, keeping memsets out of the load chain.
        # k_bf edge memsets come first (tile 0's gate the first kT/kTs
        # transposes); the big osb/v_aug memsets follow.
        kbf_tiles = []
        vaug_tiles = []
        for _ in range(5):
            t = kbf_pool.tile([P, TB + 2, D], bf16)
            nc.vector.memset(t[:, 0, :], 0.0)
            nc.vector.memset(t[:, TB + 1, :], 0.0)
            kbf_tiles.append(t)
        osb_tiles = []  # [half][buf]
        for _ in range(2):
            bufs = []
            for _ in range(2):
                t = osb_pool.tile([80, S // 2], bf16)
                # partition base must be 32-aligned; row 64 (denominators)
                # is rewritten by every head's copy
                nc.vector.memset(t[D : 80, :], 0.0)
                bufs.append(t)
            osb_tiles.append(bufs)
        for _ in range(heads):
            t2 = vaug_pool.tile([P, TB, D + 1], bf16)
            nc.vector.memset(t2[:], 1.0)
            vaug_tiles.append(t2)

        pend = {}  # head -> (q_bf, k_bf, v_aug, qT, kT, kTs) ready

        last_qT_tr = [None]  # previous head's last transpose (pacing anchor)

        def issue_loads(hd):
            PHASE_MARKS.append((nc.next_id(), f"loads_h{hd}"))
            # K first (its transposes gate the first score matmul), V last.
            # Pace copy issue so the DMA queue never jams the XBAR path:
            # loads of head h+1 and this head's V load go behind this head's
            # transposes via explicit dep edges.
            # K staging padded by one 64-col zero block on each side so the
            # shifted transposes below can cover edge t-blocks.
            k_bf = kbf_tiles[hd % len(kbf_tiles)]
            k_ld = nc.gpsimd.dma_start(
                out=k_bf[:, 1 : TB + 1, :],
                in_=k_d[hd % heads].rearrange("(n p) d -> p n d", p=P),
            )
            q_bf = qbf_pool.tile([P, TB, D], bf16)
            q_ld = nc.gpsimd.dma_start(
                out=q_bf[:], in_=q_d[hd % heads].rearrange("(n p) d -> p n d", p=P)
            )
            if last_qT_tr[0] is not None:
                add_dep_helper(k_ld.ins, last_qT_tr[0],
                               reason="pace loads behind prev transposes")
                add_dep_helper(q_ld.ins, last_qT_tr[0],
                               reason="pace loads behind prev transposes")
            # V loads for the first pass all go out upfront (see the g==0
            # block): any V load sitting behind a paced K load in the SWDGE
            # queue gets blocked by the hoisted pacing wait and starves the
            # OT matmuls during ramp-up. In reps mode, re-issue per head
            # (same data) to keep the measured steady state honest.
            v_aug = vaug_tiles[hd % heads]
            if hd >= heads:
                v_ld = nc.gpsimd.dma_start(
                    out=v_aug[:, :, 0:D],
                    in_=v_d[hd % heads].rearrange("(n p) d -> p n d", p=P),
                )
                add_dep_helper(v_ld.ins, last_qT_tr[0],
                               reason="steady-state v after transposes")
            # Natural transposes: slot m has t-block 2m on partitions 0-63 and
            # t-block 2m+1 on partitions 64-127 (k_bf block i holds t-block i-1).
            kT = kT_pool.tile([P, TB // 2, P], bf16)
            nc.sync.dma_start(out=kT[:], in_=k_bf[:, 1 : TB + 1, :], transpose=True)
            # Shifted transposes: slot m has t-block 2m-1 on partitions 0-63
            # and t-block 2m on partitions 64-127 (junk/zero at the edges).
            kTs = kTs_pool.tile([P, TB // 2 + 1, P], bf16)
            nc.sync.dma_start(out=kTs[:], in_=k_bf[:, 0 : TB + 2, :], transpose=True)
            # Batched xbar transposes: out[:, m, :] = in[:, 128m:128(m+1)].T
            qT = qT_pool.tile([P, TB // 2, P], bf16)
            qT_tr = nc.sync.dma_start(out=qT[:], in_=q_bf[:], transpose=True)
            last_qT_tr[0] = qT_tr.ins
            if dumps and hd == 0:
                nc.sync.dma_start(out=dump_d["qT_dump"][:], in_=qT[:])
                nc.sync.dma_start(out=dump_d["kT_dump"][:], in_=kT[:])
                nc.sync.dma_start(out=dump_d["kTs_dump"][:], in_=kTs[:])
            pend[hd] = (q_bf, k_bf, v_aug, qT, kT, kTs)

        def kt_block(kT, kTs, tb, bq):
            """lhsT [64, 128] for t-block tb based at partition 64*bq."""
            lo = 64 * bq
            if bq == tb % 2:
                return kT[lo : lo + 64, tb // 2, :]
            if bq == 0:  # tb odd: shifted slot (tb+1)//2, lower half
                return kTs[0:64, (tb + 1) // 2, :]
            return kTs[64:128, tb // 2, :]  # tb even, upper half

        OT_LAG = ot_lag
        TAIL_LAG = 11  # next-head score-unit where the normalize tail begins

        def emit_ot(ot_ab, v_aug, e_tiles, tb):
            vt = v_aug[:, tb, :]
            ets = e_tiles.pop(tb)
            for c in range(4):
                nc.tensor.matmul(
                    ot_ab[c // 2][:, 512 * (c % 2) : 512 * (c % 2 + 1)],
                    lhsT=vt,
                    rhs=ets[c][:],
                    start=(tb == 0),
                    stop=(tb == TB - 1),
                )

        fin_pend = {}  # head -> osT tile(s) awaiting the normalize tail

        def finish_front(hd, ot_ab, last=False):
            """O^T psum halves -> SBUF bf16 -> XBAR transpose to s-major.

            Half A copied by ACT, half B by DVE, in parallel: psOTa/b are
            freed within ~1 unit so the next head's OT accumulation starts
            on time. O^T transits bf16, which costs ~0.4% on output and the
            denominators - fine at 2e-2.
            """
            PHASE_MARKS.append((nc.next_id(), f"finfront_h{hd}"))
            osTs = []
            for h2 in range(2):
                osb = osb_tiles[h2][hd % 2]
                if h2 == 0:
                    nc.scalar.copy(osb[0 : D + 1, :], ot_ab[h2][:])
                else:
                    nc.vector.tensor_copy(osb[0 : D + 1, :], ot_ab[h2][:])
                osT = osT_pool.tile([P, TB // 2, 80], bf16)
                nc.sync.dma_start(out=osT[:], in_=osb[:], transpose=True)
                osTs.append(osT)
            fin_pend[hd] = osTs

        def finish_tail_start(hd):
            """Reciprocals of the denominators (osT col 64) on DVE."""
            PHASE_MARKS.append((nc.next_id(), f"fintail_h{hd}"))
            osT_a, osT_b = fin_pend[hd]
            rec_a = den_pool.tile([P, TB // 2], f32)
            nc.vector.reciprocal(rec_a[:], osT_a[:, :, D])
            rec_b = den_pool.tile([P, TB // 2], f32)
            nc.vector.reciprocal(rec_b[:], osT_b[:, :, D])
            obuf_a = obuf_pool.tile([P, 8, D], f32)
            obuf_b = obufb_pool.tile([P, 8, D], f32)
            fin_pend[hd] = (osT_a, osT_b, rec_a, rec_b, obuf_a, obuf_b)

        def finish_tail_piece(hd, jj):
            """Normalize block jj of each half: ACT (Copy with per-partition
            scale AP) for half A, DVE for half B. One piece per unit keeps
            the normalize work from bursting ahead of exps in the queues;
            separate obuf tiles avoid accessor serialization."""
            osT_a, osT_b, rec_a, rec_b, obuf_a, obuf_b = fin_pend[hd]
            nc.scalar.mul(
                obuf_a[:, jj, :], osT_a[:, jj, 0:D], rec_a[:, jj : jj + 1]
            )
            nc.vector.tensor_scalar_mul(
                obuf_b[:, jj, :], osT_b[:, jj, 0:D], rec_b[:, jj : jj + 1]
            )

        def finish_tail_store(hd):
            _, _, _, _, obuf_a, obuf_b = fin_pend.pop(hd)
            dst = o_d[hd % heads].rearrange("(m b p) d -> p m b d", m=8, b=2, p=P)
            nc.sync.dma_start(out=dst[:, :, 0, :], in_=obuf_a[:])
            nc.sync.dma_start(out=dst[:, :, 1, :], in_=obuf_b[:])

        # Global software-pipelined stream over (head, t-block) units.
        # Scores+exp for unit g are emitted at step g; the OT accumulation for
        # unit g-OT_LAG follows, so the OT tail of head h interleaves with the
        # first score blocks of head h+1 and ACT never waits on it.
        heads_ctx = {}  # head -> (v_aug, e_tiles, ot)
        PREFETCH = 3
        total = heads * reps

        def emit_unit(g):
            hd, tb = divmod(g, TB)
            if tb == 0:
                PHASE_MARKS.append((nc.next_id(), f"score_h{hd}"))
                _q_bf, _k_bf, v_aug, qT, kT, kTs = pend.pop(hd)
                heads_ctx[hd] = {"v": v_aug, "qT": qT, "kT": kT, "kTs": kTs,
                                 "e": {}, "ot": None}
            ctx_h = heads_ctx[hd]
            qT, kT, kTs = ctx_h["qT"], ctx_h["kT"], ctx_h["kTs"]
            # Exp is split across ScalarE (native) and VectorE (Schraudolph),
            # each issued right after the 512-col matmul filling its psum
            # bank, so the psum WAR chains resolve well before the next
            # unit's score matmuls need the banks.
            e_tiles = []
            for q4 in range(4):
                bq, g2 = divmod(q4, 2)
                st = psQ[q4].tile([P, 512], f32)
                nc.tensor.matmul(
                    st[:],
                    lhsT=kt_block(kT, kTs, tb, bq),
                    rhs=qT[64 * bq : 64 * bq + 64, 4 * g2 : 4 * g2 + 4, :],
                    start=True,
                    stop=True,
                )
                if g2 == 0:  # ScalarE native exp
                    et = e_act_pools[bq].tile([P, 512], bf16)
                    nc.scalar.activation(et[:], st[:], EXP)
                else:  # VectorE Schraudolph
                    et = e_dve_pools[bq].tile([P, 512], bf16)
                    nc.vector.tensor_scalar(
                        et[:].bitcast(i16), st[:], SCHR_A, SCHR_B, MULT, ADD,
                    )
                e_tiles.append(et)
            if dumps and hd == 0:
                for q4 in range(4):
                    nc.sync.dma_start(
                        out=dump_d["e_dump"][tb, :, 512 * q4 : 512 * (q4 + 1)],
                        in_=e_tiles[q4][:],
                    )
            ctx_h["e"][tb] = e_tiles
            if hd >= 1 and (hd - 1) in fin_pend:
                if tb == TAIL_LAG:
                    finish_tail_start(hd - 1)
                elif TAIL_LAG < tb <= TAIL_LAG + 4:
                    jj0 = 2 * (tb - TAIL_LAG - 1)
                    finish_tail_piece(hd - 1, jj0)
                    finish_tail_piece(hd - 1, jj0 + 1)
                    if tb == TAIL_LAG + 4:
                        finish_tail_store(hd - 1)

        def emit_ot_unit(g):
            hd, tb = divmod(g, TB)
            ctx_h = heads_ctx[hd]
            if tb == 0:
                ot_a = psOTa.tile([D + 1, S // 2], f32, tag="otslotA")
                ot_b = psOTb.tile([D + 1, S // 2], f32, tag="otslotB")
                ctx_h["ot"] = (ot_a, ot_b)
            emit_ot(ctx_h["ot"], ctx_h["v"], ctx_h["e"], tb)
            if tb == TB - 1:
                finish_front(hd, ctx_h["ot"], last=(hd == total - 1))
                heads_ctx.pop(hd)

        n_units = total * TB
        for g in range(n_units + OT_LAG):
            gh = g // TB
            if g == 0:
                issue_loads(0)
                # All first-pass V tensors upfront, before any paced K load
                # enters the SWDGE queue (a hoisted pacing wait would block
                # them and starve the OT matmuls during ramp-up).
                for vh in range(heads):
                    nc.gpsimd.dma_start(
                        out=vaug_tiles[vh][:, :, 0:D],
                        in_=v_d[vh].rearrange("(n p) d -> p n d", p=P),
                    )
                for pf in range(1, min(PREFETCH, total)):
                    issue_loads(pf)
                # warm the exp table set while the first input DMAs are in
                # flight
                nc.vector.memset(warm[:], 0.0)
                nc.scalar.activation(warm[:], warm[:], EXP)
            elif g % TB == 0 and gh + PREFETCH - 1 < total:
                issue_loads(gh + PREFETCH - 1)
            if g < n_units:
                emit_unit(g)
            if g >= OT_LAG:
                emit_ot_unit(g - OT_LAG)
        # last head's tail: nothing left to interleave with, emit as a burst
        finish_tail_start(total - 1)
        for jj in range(TB // 2):
            finish_tail_piece(total - 1, jj)
        finish_tail_store(total - 1)

    PHASE_MARKS.append((nc.next_id(), "END"))
    if not nc.is_finalized():
        nc.finalize()
    return nc


def _get_program():
    if "nc" not in _prog_cache:
        _prog_cache["nc"] = _build_program()
    return _prog_cache["nc"]


def _run(in_maps, trace=False):
    if _TRN_REPO not in sys.path:
        sys.path.insert(0, _TRN_REPO)
    from concourse.bass_utils import run_bass_kernel_spmd

    nc = _get_program()
    return run_bass_kernel_spmd(nc, in_maps, list(range(N_CORES)), trace=trace)


def _make_in_maps(input_query, input_key, input_value):
    q = np.ascontiguousarray(np.asarray(input_query, np.float32)).reshape(B * H, S, D)
    k = np.ascontiguousarray(np.asarray(input_key, np.float32)).reshape(B * H, S, D)
    v = np.ascontiguousarray(np.asarray(input_value, np.float32)).reshape(B * H, S, D)
    in_maps = []
    for i in range(N_CORES):
        sl = slice(i * HEADS, (i + 1) * HEADS)
        in_maps.append(
            {
                "q": np.ascontiguousarray(q[sl]),
                "k": np.ascontiguousarray(k[sl]),
                "v": np.ascontiguousarray(v[sl]),
            }
        )
    return in_maps


def kernel(input_query, input_key, input_value):
    in_maps = _make_in_maps(input_query, input_key, input_value)
    res = _run(in_maps, trace=False)
    out = np.concatenate([np.asarray(r["out"]) for r in res.results], axis=0)
    return out.reshape(B, H, S, D).astype(np.float32)


def kernel_traced(input_query, input_key, input_value):
    """Like kernel() but with neuron-profile tracing; returns (out, results)."""
    in_maps = _make_in_maps(input_query, input_key, input_value)
    res = _run(in_maps, trace=True)
    out = np.concatenate([np.asarray(r["out"]) for r in res.results], axis=0)
    return out.reshape(B, H, S, D).astype(np.float32), res

